# revision 24
# baseline (speedup 1.0000x reference)
"""NetVLAD (vq_codebook) Trainium2 Bass kernel, 8-way spatially sharded. v3

Per-core math (x pre-normalized on host; fp8 for the logits stream,
bf16 for the VLAD stream):
  logits[k,l] = cwt^T xn  (PE, fp8->f32)        [K=64, Ls=4992]
  logitsT via xbar DMA transpose -> exp (ACT) -> top-2 keep (DVE)
  cnt = 3x3 box-sum of keep (DVE, separable)    needs [K, L] layout
  w2 = cnt * exp * mask^4 / sumexp              [l, k] layout
  vlad[k,c] = sum_l w2 xn  (PE, bf16)           + sa2sum via add-tree+MM

Layout trick: the 26-row slab (4992 loc) is split into two halves A (rows
0..13, tiles 0..20) and B (rows 12..25, tiles 18..38) that overlap by 2
rows; A's k-planes live on partitions 0..63 and B's on 64..127 of one
packed [128, 2688] buffer, so the box-sum runs at full partition width.
Valid outputs split cleanly at row 12/13 (sc0A/sc0B masks); tile 19
straddles the boundary and gets a small seam fix-up.

All [L,K]<->[K,L] transposes ride the xbar DMA-transpose engine on the
Activation HWDGE ring (zero PE cost); input streams ride the SP ring.
Per-slot scalars (row maxes, scc) are materialized to full width on the
ACT engine so every DVE compare/multiply runs dense bf16 (2x mode).
"""
import os
import sys

sys.path.insert(0, "/opt/trn_rl_repo")
os.environ.setdefault("MYCRO_LOCAL_CACHE", "1")

import numpy as np

C, H, W, K = 512, 192, 192, 64
M = 8                      # cores
RPC = H // M               # 24 rows per core
Ls = (RPC + 2) * W         # 4992 slab locations (incl. 1 halo row each side)
NT = Ls // 128             # 39 l-tiles
CT = C // 128              # 4 c-tiles
S = 21                     # slots per half
OV = 18                    # B half starts at tile 18
HW2 = S * 128              # 2688 locations per half
G2 = 320                   # guard cols (32B-aligned for xbar, even parity)
KBW = G2 + HW2 + G2        # 3328 packed keep-buffer width
XW = 8                     # xlcn DMA wave size (tiles)
DCH = 8                    # xcl DMA chunks
SK = S * K

TRACE = False              # set by test.py for profiling runs
_CACHE = {}


def _build_nc():
    import concourse.bass as bass
    import concourse.tile as tile
    from concourse import mybir

    f32 = mybir.dt.float32
    bf16 = mybir.dt.bfloat16
    fp8 = mybir.dt.float8e4
    AF = mybir.ActivationFunctionType
    OP = mybir.AluOpType
    AX = mybir.AxisListType
    PF = mybir.PoolFunctionType

    nc = bass.Bass()
    xcl = nc.dram_tensor("xcl", [C, Ls], fp8, kind="ExternalInput")
    xlcn = nc.dram_tensor("xlcn", [Ls, C], bf16, kind="ExternalInput")
    cwt = nc.dram_tensor("cwt", [C, K], fp8, kind="ExternalInput")
    sc0a = nc.dram_tensor("sc0a", [128, S], f32, kind="ExternalInput")
    sc0b = nc.dram_tensor("sc0b", [128, S], f32, kind="ExternalInput")
    ones = nc.dram_tensor("ones", [128, 128], bf16, kind="ExternalInput")
    y = nc.dram_tensor("y", [K, C + 1], f32, kind="ExternalOutput")

    with tile.TileContext(nc) as tc:
        with tc.tile_pool(name="big", bufs=1) as big:
            xcl_sb = big.tile([128, CT * Ls], fp8, tag="xcl")
            xlcn_sb = big.tile([128, NT * C], bf16, tag="xlcn")
            logkl = big.tile([K, Ls], bf16, tag="logkl")
            logT = big.tile([128, 2 * SK], bf16, tag="logT")
            expb = big.tile([128, 2 * SK], bf16, tag="expb")
            t3b = big.tile([128, 2 * SK], bf16, tag="t3b")
            u3b = big.tile([128, SK], bf16, tag="u3b")
            mwx = big.tile([128, SK], bf16, tag="mwx")
            mws = big.tile([128, SK], bf16, tag="mws")
            kp = big.tile([128, S * 128], bf16, tag="kp")
            kbp = big.tile([128, KBW], bf16, tag="kbp")
            h3s = big.tile([128, KBW], bf16, tag="h3s")
            cntp = big.tile([128, S * 128], bf16, tag="cntp")
            cntT = big.tile([128, S * 128], bf16, tag="cntT")
            w2b = big.tile([128, NT * K], bf16, tag="w2b")
            tw = big.tile([128, 2 * SK], bf16, tag="tw")
            sseam = big.tile([128, 3 * K], bf16, tag="sseam")
            cwt_sb = big.tile([128, CT * K], fp8, tag="cwt")
            ones_sb = big.tile([128, 128], bf16, tag="ones")
            sca_sb = big.tile([128, S], f32, tag="sc0a")
            scb_sb = big.tile([128, S], f32, tag="sc0b")
            sume = big.tile([128, 2 * S], f32, tag="sume")
            isum = big.tile([128, 2 * S], f32, tag="isum")
            m1b = big.tile([128, 2 * S], f32, tag="m1b")
            m2b = big.tile([128, 2 * S], f32, tag="m2b")
            scc = big.tile([128, 2 * S], f32, tag="scc")
            s1b = big.tile([128, K], bf16, tag="s1b")
            vl_sb = big.tile([K, C + 1], f32, tag="vl")
            scr = big.tile([128, 4], f32, tag="scr")

            # ---- input DMAs (SP ring): cwt + first xcl chunk first (the
            # sync queue spends ~0.65us issuing each dma_start, so the
            # logits-critical transfers go to the head of the line)
            xc3 = xcl[:].rearrange("(ct p) l -> p ct l", p=128)
            xs3 = xcl_sb[:].rearrange("p (ct l) -> p ct l", l=Ls)
            csz = Ls // DCH
            nc.sync.dma_start(
                cwt_sb[:].rearrange("p (t k) -> p t k", k=K),
                cwt[:].rearrange("(t p) k -> p t k", p=128),
            )
            for j in range(DCH):
                nc.sync.dma_start(
                    xs3[:, :, j * csz:(j + 1) * csz],
                    xc3[:, :, j * csz:(j + 1) * csz],
                )
            nc.sync.dma_start(ones_sb[:], ones[:])
            nc.sync.dma_start(sca_sb[:], sc0a[:])
            nc.sync.dma_start(scb_sb[:], sc0b[:])
            x3 = xlcn[:].rearrange("(a p) c -> p a c", p=128)
            NW = (NT + XW - 1) // XW
            for w in range(NW):
                n = min(XW, NT - w * XW)
                nc.sync.dma_start(
                    xlcn_sb[:, w * XW * C:(w * XW + n) * C].rearrange(
                        "p (a c) -> p a c", c=C),
                    x3[:, w * XW:w * XW + n, :],
                )

            # zero keep-buffer guards; absorb small-input DMA completions
            nc.vector.memset(kbp[:, 0:G2], 0.0)
            nc.vector.memset(kbp[:, G2 + HW2:KBW], 0.0)
            nc.vector.tensor_copy(scr[:, 0:1], sca_sb[:, 0:1])
            nc.vector.tensor_copy(scr[:, 1:2], scb_sb[:, 0:1])

            with tc.tile_pool(name="pp", bufs=1, space="PSUM") as pp:
                pv0 = pp.tile([K, C], f32, tag="pv0", bufs=1)
                ps1 = pp.tile([K, 1], f32, tag="ps1", bufs=1)
                # PE warm-up; absorbs the cwt DMA wait, then a long burst
                # of 512-row dummies keeps the clock ramping while the first
                # xcl chunks stream in
                dummy = pp.tile([K, 512], f32, tag="plk", bufs=4)
                nc.tensor.matmul(dummy[0:64, 0:64], lhsT=cwt_sb[:, 0:64],
                                 rhs=cwt_sb[:, 0:64], start=True, stop=True)
                for _ in range(6):
                    dummy = pp.tile([K, 512], f32, tag="plk", bufs=4)
                    nc.tensor.matmul(dummy[0:64, 0:64], lhsT=cwt_sb[:, 0:64],
                                     rhs=cwt_sb[:, 0:64], start=True, stop=True)

                # ---- phase 1: logits [K, L] in 512-col blocks ----
                nblk = (Ls + 511) // 512
                touched = set()
                for b in range(nblk):
                    w = min(512, Ls - b * 512)
                    for j in range((b * 512) // csz,
                                   (b * 512 + w - 1) // csz + 1):
                        if j not in touched:
                            touched.add(j)
                            dj = pp.tile([K, 512], f32, tag="plk", bufs=4)
                            nc.tensor.matmul(
                                dj[0:64, 0:64],
                                lhsT=xcl_sb[:, j * csz:j * csz + 64],
                                rhs=xcl_sb[:, j * csz:j * csz + 64],
                                start=True, stop=True)
                    plk = pp.tile([K, 512], f32, tag="plk", bufs=4)
                    for ct in range(CT):
                        nc.tensor.matmul(
                            plk[:, 0:w],
                            lhsT=cwt_sb[:, ct * K:(ct + 1) * K],
                            rhs=xcl_sb[:, ct * Ls + b * 512:
                                       ct * Ls + b * 512 + w],
                            start=(ct == 0),
                            stop=(ct == CT - 1),
                        )
                    nc.scalar.copy(logkl[:, b * 512:b * 512 + w],
                                   plk[:, 0:w])
                    if b < 8:
                        # keep the PE clock ramped while DMA-gated
                        dw = pp.tile([K, 512], f32, tag="plk", bufs=4)
                        nc.tensor.matmul(
                            dw[0:64, :], lhsT=cwt_sb[:, 0:64],
                            rhs=xcl_sb[:, 0:512], start=True, stop=True)
                    # xbar transposes (ACT HWDGE ring) per half
                    if b == 5:
                        nc.scalar.dma_start_transpose(
                            logT[:, 0:SK].rearrange("p (i k) -> p i k", k=K),
                            logkl[:, 0:HW2],
                        )
                    if b == nblk - 1:
                        nc.scalar.dma_start_transpose(
                            logT[:, SK:2 * SK].rearrange(
                                "p (i k) -> p i k", k=K),
                            logkl[:, OV * 128:OV * 128 + HW2],
                        )

                # views per half
                def hv(buf, h):
                    return buf[:, h * SK:(h + 1) * SK].rearrange(
                        "p (i k) -> p i k", k=K)

                kp4 = kp[:].rearrange("p (i q) -> p i q", q=128)
                u4 = u3b[:].rearrange("p (i k) -> p i k", k=K)
                mx4 = mwx[:].rearrange("p (i k) -> p i k", k=K)
                ms4 = mws[:].rearrange("p (i k) -> p i k", k=K)

                # ---- phase 2: exp (ACT) + top-2 keep (DVE), per half ----
                nc.scalar.activation(expb[:, 0:SK], logT[:, 0:SK], AF.Exp)
                nc.scalar.activation(expb[:, SK:2 * SK], logT[:, SK:2 * SK],
                                     AF.Exp)
                eh = [hv(expb, 0), hv(expb, 1)]
                th = [hv(t3b, 0), hv(t3b, 1)]
                m1c, m2c = [], []
                for h in range(2):
                    ssl = slice(h * S, (h + 1) * S)
                    m1c.append(m1b[:, ssl][:, :, None]
                               .broadcast_to([128, S, K]))
                    m2c.append(m2b[:, ssl][:, :, None]
                               .broadcast_to([128, S, K]))
                    lh = hv(logT, h)
                    nc.vector.tensor_reduce(m1b[:, ssl], lh, axis=AX.X,
                                            op=OP.max)
                    nc.vector.tensor_tensor(th[h], lh, m1c[h], op=OP.is_ge)
                    nc.vector.scalar_tensor_tensor(
                        u4, th[h], -30.0, lh, op0=OP.mult, op1=OP.add)
                    nc.vector.tensor_reduce(m2b[:, ssl], u4, axis=AX.X,
                                            op=OP.max)
                # keep, split at slot 12 so the packed transpose + box-sum
                # pipeline in two chunks (J1 = slots 0..11, J2 = 12..20)
                J1 = 12
                lhv = [hv(logT, 0), hv(logT, 1)]
                for h in range(2):
                    nc.vector.tensor_tensor(
                        kp4[:, 0:J1, h * K:(h + 1) * K], lhv[h][:, 0:J1, :],
                        m2c[h][:, 0:J1, :], op=OP.is_ge)
                # T-keep1 fires here (ACT ring, after the two J1 is_ge ops)
                for h in range(2):
                    nc.vector.tensor_tensor(
                        kp4[:, J1:S, h * K:(h + 1) * K], lhv[h][:, J1:S, :],
                        m2c[h][:, J1:S, :], op=OP.is_ge)
                for h in range(2):
                    ssl = slice(h * S, (h + 1) * S)
                    nc.vector.tensor_reduce(sume[:, ssl], eh[h], axis=AX.X,
                                            op=OP.add)
                nc.vector.reciprocal(isum[:], sume[:])
                nc.vector.tensor_mul(scc[:, 0:S], sca_sb[:], isum[:, 0:S])
                nc.vector.tensor_mul(scc[:, S:2 * S], scb_sb[:],
                                     isum[:, S:2 * S])

                # ---- phase 3/4/5 chunked: keep -> [K, L] -> box -> back.
                # ACT ring order: T-keep1, sccwA, T-keep2, sccwB, T-cnt1,
                # T-cnt2 (each waits only its DVE producer tick).
                J1C = J1 * 128                      # 1536 kb cols in chunk 1
                V1 = 1280                           # cnt cols in chunk 1
                nc.scalar.dma_start_transpose(
                    kbp[:, G2:G2 + J1C].rearrange("p (i q) -> p i q", q=128),
                    kp[:, 0:J1C],
                )
                nc.scalar.copy(
                    mx4, scc[:, 0:S][:, :, None].broadcast_to([128, S, K]))
                nc.scalar.dma_start_transpose(
                    kbp[:, G2 + J1C:G2 + HW2].rearrange(
                        "p (i q) -> p i q", q=128),
                    kp[:, J1C:HW2],
                )
                nc.scalar.copy(
                    ms4, scc[:, S:2 * S][:, :, None].broadcast_to([128, S, K]))

                # box chunk 1: h over [G2-194, G2+1474), v over cnt [0, 1280)
                h1s, h1e = G2 - 194, G2 + V1 + 194
                nc.vector.tensor_add(
                    h3s[:, h1s:h1e], kbp[:, h1s - 1:h1e - 1],
                    kbp[:, h1s + 1:h1e + 1])
                nc.vector.tensor_add(
                    h3s[:, h1s:h1e], h3s[:, h1s:h1e], kbp[:, h1s:h1e])
                nc.vector.tensor_add(
                    cntp[:, 0:V1], h3s[:, G2 - 192:G2 - 192 + V1],
                    h3s[:, G2 + 192:G2 + 192 + V1])
                nc.vector.tensor_add(
                    cntp[:, 0:V1], cntp[:, 0:V1], h3s[:, G2:G2 + V1])
                nc.scalar.dma_start_transpose(
                    cntT[:, 0:V1].rearrange("p (i q) -> p i q", q=128),
                    cntp[:, 0:V1],
                )
                # tw = exp * scc (dense; overlaps the T-cnt1 flight)
                nc.vector.tensor_mul(hv(tw, 0), eh[0], mx4)
                nc.vector.tensor_mul(hv(tw, 1), eh[1], ms4)

                c4 = cntT[:].rearrange("p (i q) -> p i q", q=128)
                twa = hv(tw, 0)
                twb = hv(tw, 1)
                w3 = w2b[:].rearrange("p (t k) -> p t k", k=K)
                # w2 chunk 1: A tiles 0..9 and B tiles 20..27
                nc.vector.tensor_mul(
                    w3[:, 0:10, :], twa[:, 0:10, :], c4[:, 0:10, 0:K])
                nc.vector.tensor_mul(
                    w3[:, 20:28, :], twb[:, 2:10, :], c4[:, 2:10, K:128])

                # box chunk 2 + T-cnt2
                h2s, h2e = h1e, G2 + HW2 + 194
                nc.vector.tensor_add(
                    h3s[:, h2s:h2e], kbp[:, h2s - 1:h2e - 1],
                    kbp[:, h2s + 1:h2e + 1])
                nc.vector.tensor_add(
                    h3s[:, h2s:h2e], h3s[:, h2s:h2e], kbp[:, h2s:h2e])
                nc.vector.tensor_add(
                    cntp[:, V1:HW2], h3s[:, G2 - 192 + V1:G2 - 192 + HW2],
                    h3s[:, G2 + 192 + V1:G2 + 192 + HW2])
                nc.vector.tensor_add(
                    cntp[:, V1:HW2], cntp[:, V1:HW2],
                    h3s[:, G2 + V1:G2 + HW2])
                nc.scalar.dma_start_transpose(
                    cntT[:, V1:HW2].rearrange("p (i q) -> p i q", q=128),
                    cntp[:, V1:HW2],
                )

                # ---- phase 7a: VLAD wave 1 (tiles 0..9, 20..27) ----
                # warm-up burst while T-cnt1 flies (gated on box chunk 1)
                for _ in range(6):
                    dv = pp.tile([K, 512], f32, tag="plk", bufs=4)
                    nc.tensor.matmul(dv[0:64, :], lhsT=ones_sb[:, 0:64],
                                     rhs=cntp[:, 0:512], start=True, stop=True)
                for wv in range(4):
                    dv = pp.tile([K, 512], f32, tag="plk", bufs=4)
                    nc.tensor.matmul(
                        dv[0:64, 0:64],
                        lhsT=xlcn_sb[:, wv * XW * C:wv * XW * C + 64],
                        rhs=xlcn_sb[:, wv * XW * C:wv * XW * C + 64],
                        start=True, stop=True)
                dv = pp.tile([K, 512], f32, tag="plk", bufs=4)
                nc.tensor.matmul(dv[0:64, 0:64],
                                 lhsT=w2b[:, 20 * K:20 * K + 64],
                                 rhs=w2b[:, 20 * K:20 * K + 64],
                                 start=True, stop=True)
                for t in list(range(0, 10)) + list(range(20, 28)):
                    nc.tensor.matmul(
                        pv0[:], lhsT=w2b[:, t * K:(t + 1) * K],
                        rhs=xlcn_sb[:, t * C:(t + 1) * C],
                        start=(t == 0), stop=False)

                # w2 chunk 2: A tiles 10..19, B tiles 28..38, seam on tile 19
                nc.vector.tensor_mul(
                    w3[:, 10:20, :], twa[:, 10:20, :], c4[:, 10:20, 0:K])
                nc.vector.tensor_mul(
                    w3[:, 28:NT, :], twb[:, 10:S, :], c4[:, 10:S, K:128])
                nc.vector.tensor_mul(
                    sseam[:, 0:K], twb[:, 1, :], c4[:, 1, K:128])
                nc.vector.tensor_add(
                    w3[:, 19, :], w3[:, 19, :], sseam[:, 0:K])

                # ---- phase 7b: VLAD wave 2 (tiles 10..19, 28..38) ----
                for wv in range(4, NW):
                    dv = pp.tile([K, 512], f32, tag="plk", bufs=4)
                    nc.tensor.matmul(
                        dv[0:64, 0:64],
                        lhsT=xlcn_sb[:, wv * XW * C:wv * XW * C + 64],
                        rhs=xlcn_sb[:, wv * XW * C:wv * XW * C + 64],
                        start=True, stop=True, skip_group_check=True)
                dv = pp.tile([K, 512], f32, tag="plk", bufs=4)
                nc.tensor.matmul(dv[0:64, 0:64],
                                 lhsT=w2b[:, 19 * K:19 * K + 64],
                                 rhs=w2b[:, 19 * K:19 * K + 64],
                                 start=True, stop=True, skip_group_check=True)
                for t in list(range(10, 20)) + list(range(28, NT)):
                    nc.tensor.matmul(
                        pv0[:], lhsT=w2b[:, t * K:(t + 1) * K],
                        rhs=xlcn_sb[:, t * C:(t + 1) * C],
                        start=False, stop=(t == NT - 1))

                # sa2 row-sums: dense bf16 add-tree over the 39 tiles
                # (t3b is free scratch by now), then one ones-matmul
                nc.vector.tensor_add(
                    t3b[:, 0:1024], w2b[:, 0:1024], w2b[:, 1024:2048])
                nc.vector.tensor_add(
                    t3b[:, 0:448], t3b[:, 0:448], w2b[:, 2048:2496])
                nc.vector.tensor_add(
                    t3b[:, 1024:1536], t3b[:, 0:512], t3b[:, 512:1024])
                nc.vector.tensor_add(
                    t3b[:, 1536:1792], t3b[:, 1024:1280], t3b[:, 1280:1536])
                nc.vector.tensor_add(
                    t3b[:, 1792:1920], t3b[:, 1536:1664], t3b[:, 1664:1792])
                nc.vector.tensor_add(
                    s1b[:], t3b[:, 1792:1856], t3b[:, 1856:1920])
                nc.tensor.matmul(ps1[:], lhsT=s1b[:], rhs=ones_sb[:, 0:1],
                                 start=True, stop=True)

                # ---- phase 8: write partial sums ----
                nc.scalar.copy(vl_sb[:, 0:C], pv0[:])
                nc.scalar.copy(vl_sb[:, C:C + 1], ps1[:])
                nc.sync.dma_start(y[:], vl_sb[:])
    _prune_waits(nc)
    return nc


def _prune_waits(nc):
    """Drop semaphore waits that are transitively implied by another wait on
    the same instruction (the walrus codegen allows at most ONE sync wait
    per instruction).

    Per-proc completion is in-order (engine FIFOs, per-queue DMA), so
    "sem S reached v" implies all waits of every instruction on S's proc
    with cumulative tick <= v held.  Waits of non-updating instructions
    (e.g. InstLdweights) are attributed to the next same-engine updater.
    """
    insts = [ins for bb in nc.main_func.blocks for ins in bb.instructions]
    # The xbar-transpose dst is lowered through an aliased AP whose encoded
    # stride makes its address range falsely overlap unrelated buffers, so
    # Tile hangs DMA-completion waits on the transposes (e.g. T-keep waiting
    # for an xlcn wave).  None of the transposes' true sources or sinks are
    # DMA-written (they are ACT/DVE-produced SBUF tensors), and same-ring
    # ordering is FIFO in hardware, so every DMAHW wait on them is vacuous.
    for ins in insts:
        if type(ins).__name__ == "InstDmaTransposeAnt":
            si = getattr(ins, "sync_info", None)
            if si is not None and si.on_wait:
                si.on_wait = [
                    w for w in si.on_wait
                    if not w.ant_name.startswith("DMAHW")
                ]
    proc_events = {}
    waits_of = {}
    pending_by_engine = {}
    for ins in insts:
        si = getattr(ins, "sync_info", None)
        if si is None:
            continue
        eng = getattr(ins, "engine", None)
        ow = [(w.ant_name, w.wait_value) for w in (si.on_wait or [])]
        ups = [
            u for u in (si.on_update or [])
            if getattr(u, "update_mode", None) in ("sem-inc", "sem-add-imm")
        ]
        if ups:
            merged = pending_by_engine.pop(eng, [])
            merged.extend(ow)
            waits_of[id(ins)] = merged
            for u in ups:
                lst = proc_events.setdefault(u.ant_name, [])
                prev = lst[-1][0] if lst else 0
                lst.append((prev + (u.update_value or 1), ins))
        else:
            waits_of[id(ins)] = ow
            if eng is not None and ow:
                pending_by_engine.setdefault(eng, []).extend(ow)

    import bisect

    def prefix_index(sem, v):
        lst = proc_events.get(sem)
        if not lst:
            return None
        ticks = [t for t, _ in lst]
        i = bisect.bisect_left(ticks, v)
        return i if i < len(lst) else None

    memo = {}
    in_progress = object()

    def holds(sem, v):
        """Thresholds guaranteed held once sem >= v."""
        i = prefix_index(sem, v)
        if i is None:
            return {}
        key = (sem, i)
        got = memo.get(key)
        if got is in_progress:
            return {}       # cycle: under-approximate, do NOT memoize
        if got is not None:
            return got
        memo[key] = in_progress
        out = {}
        inorder = not sem.startswith("Pool")
        rng = range(i + 1) if inorder else (i,)
        for j in rng:
            _, ins = proc_events[sem][j]
            for (s2, v2) in waits_of.get(id(ins), []):
                if out.get(s2, 0) < v2:
                    out[s2] = v2
                sub = holds(s2, v2)
                for s3, v3 in sub.items():
                    if out.get(s3, 0) < v3:
                        out[s3] = v3
        memo[key] = out
        return out

    own_tick = {}
    for sem, lst in proc_events.items():
        for tick, ins in lst:
            own_tick[(id(ins), sem)] = tick

    pruned = 0
    for ins in insts:
        si = getattr(ins, "sync_info", None)
        if si is None or not si.on_wait or len(si.on_wait) < 2:
            continue
        ow = list(si.on_wait)
        kept = list(ow)
        for w in ow:
            if len(kept) == 1:
                break
            mine = own_tick.get((id(ins), w.ant_name))
            if mine is not None and w.wait_value <= mine - 1:
                kept.remove(w)
                pruned += 1
                continue
            others = [o for o in kept if o is not w]
            for o in others:
                h = holds(o.ant_name, o.wait_value)
                if h.get(w.ant_name, 0) >= w.wait_value:
                    kept.remove(w)
                    pruned += 1
                    break
        si.on_wait = kept
    leftovers = [
        (getattr(ins, "name", "?"),
         [(x.ant_name, x.wait_value) for x in ins.sync_info.on_wait])
        for ins in insts
        if getattr(ins, "sync_info", None) is not None
        and ins.sync_info.on_wait and len(ins.sync_info.on_wait) > 1
    ]
    assert not leftovers, f"multi-wait instructions survive prune: {leftovers}"
    return pruned


def _host_prep(x, conv_w, centroids):
    from concourse import mybir
    bf16np = mybir.dt.np(mybir.dt.bfloat16)
    fp8np = mybir.dt.np(mybir.dt.float8e4)

    x = np.ascontiguousarray(x, dtype=np.float32)
    L = H * W
    norm = np.sqrt((x.astype(np.float64) ** 2).sum(0))
    norm = np.maximum(norm, 1e-12).astype(np.float32)       # [H,W]
    xn = x / norm
    ii = np.arange(H, dtype=np.float32)
    mi = np.minimum(H - 1 - ii, ii)
    m = np.minimum(mi[:, None], mi[None, :]).astype(np.float32)
    m2 = m * m
    mask4 = m2 * m2                                          # [H,W]

    xpad = np.zeros((C, H + 2, W), fp8np)
    xpad[:, 1:H + 1, :] = xn.astype(fp8np)
    xtn = np.zeros(((H + 2) * W, C), bf16np)
    xtn[W:(H + 1) * W, :] = xn.reshape(C, L).T.astype(bf16np)
    mask_pad = np.zeros((H + 2) * W, np.float32)
    mask_pad[W:(H + 1) * W] = mask4.reshape(L)

    cwt = np.ascontiguousarray(conv_w.T).astype(fp8np)       # [C,K]
    ones = np.ones((128, 128), bf16np)

    lrow = np.arange(Ls) // W                                # slab row of l
    in_maps = []
    for core in range(M):
        r0 = core * RPC
        sl = slice(r0 * W, (r0 + RPC + 2) * W)
        mc = mask_pad[sl].copy()
        mc[0:W] = 0.0                                        # halo rows
        mc[(RPC + 1) * W:] = 0.0
        # A half: tiles 0..20, valid rows 1..12; B: tiles 18..38, rows 13..24
        sc0A = np.zeros((S, 128), np.float32)
        sc0B = np.zeros((S, 128), np.float32)
        for i in range(S):
            la = i * 128 + np.arange(128)
            sc0A[i] = np.where(lrow[la] <= 12, mc[la], 0.0)
            lb = (OV + i) * 128 + np.arange(128)
            sc0B[i] = np.where(lrow[lb] >= 13, mc[lb], 0.0)
        in_maps.append({
            "xcl": np.ascontiguousarray(
                xpad[:, r0:r0 + RPC + 2, :].reshape(C, Ls)),
            "xlcn": np.ascontiguousarray(xtn[sl]),
            "cwt": cwt,
            "sc0a": np.ascontiguousarray(sc0A.T),
            "sc0b": np.ascontiguousarray(sc0B.T),
            "ones": ones,
        })
    return in_maps


def _ensure_ntff_hook():
    """Install the axon NTFF profile hook if the image's antenv lacks it."""
    import types
    try:
        from antenv.axon_hooks import get_axon_ntff_profile_hook  # noqa: F401
        return
    except ImportError:
        pass
    if "/root/.axon_site" not in sys.path:
        sys.path.insert(0, "/root/.axon_site")
    from trn_agent_boot.trn_boot import _ntff_profile_via_ctypes
    hook = _ntff_profile_via_ctypes("/opt/axon/libaxon_pjrt.so")
    mod = types.ModuleType("antenv.axon_hooks")
    mod.get_axon_ntff_profile_hook = lambda: hook
    mod.set_axon_ntff_profile_hook = lambda h: None
    import antenv
    antenv.axon_hooks = mod
    sys.modules["antenv.axon_hooks"] = mod


def _install_neff_cache():
    """Cache compiled NEFFs across processes, keyed by BIR content hash."""
    import hashlib
    import shutil
    import concourse.bass2jax as b2j

    orig = b2j.compile_bir_kernel
    if getattr(orig, "_neff_cached", False):
        return

    def cached(bir_json, tmpdir, neff_name="file.neff"):
        h = hashlib.sha256(
            bir_json if isinstance(bir_json, bytes) else bir_json.encode()
        ).hexdigest()[:24]
        cdir = "/tmp/neff_cache"
        os.makedirs(cdir, exist_ok=True)
        cpath = os.path.join(cdir, h + ".neff")
        if os.path.exists(cpath):
            dst = os.path.join(tmpdir, neff_name)
            os.makedirs(tmpdir, exist_ok=True)
            shutil.copy(cpath, dst)
            return dst
        out = orig(bir_json, tmpdir, neff_name=neff_name)
        shutil.copy(out, cpath)
        return out

    cached._neff_cached = True
    b2j.compile_bir_kernel = cached


def kernel(x, conv_w, centroids):
    import concourse.bass_utils as bu
    from concourse.bass_utils import run_bass_kernel_spmd
    _install_neff_cache()
    if TRACE:
        _ensure_ntff_hook()
        bu.upload_artifacts = lambda tmpdir: "local://" + tmpdir

    if "nc" not in _CACHE:
        _CACHE["nc"] = _build_nc()
    nc = _CACHE["nc"]
    in_maps = _host_prep(np.asarray(x), np.asarray(conv_w), np.asarray(centroids))
    res = run_bass_kernel_spmd(nc, in_maps, list(range(M)), trace=TRACE)
    _CACHE["last"] = res
    red = np.zeros((K, C + 1), np.float32)
    for r in res.results:
        red += np.asarray(r["y"], dtype=np.float32)
    vlad = red[:, :C] - red[:, C:C + 1] * np.asarray(centroids, np.float32)
    vlad /= np.maximum(np.sqrt((vlad ** 2).sum(1))[:, None], 1e-12)
    v = vlad.reshape(1, K * C)
    v /= np.maximum(np.sqrt((v ** 2).sum()), 1e-12)
    return v.astype(np.float32)
